# revision 55
# baseline (speedup 1.0000x reference)
"""Trainium2 Bass/Tile kernel: fused fp8-quantized multi-head causal attention.

Module: q/k/v = fp8(x) @ fp8(W) + b ; scores = (q k^T)/sqrt(64) with causal
mask (-1000 => exp underflows to exactly 0) ; out = softmax(scores) @ v @ W_O + b_O.

Sharding (8 NeuronCores, SPMD, no collectives):
  core c -> batch b = c // 4, head group hg = c % 4 (heads 4*hg .. 4*hg+3).
  Each core returns a partial [S, M] bf16 output (its 4 heads' contribution);
  the host sums the 4 partials per batch in f32 and adds b_O + sum_h b_V[h]@W_O[h]
  (exact: softmax rows sum to 1, so the V bias contributes a constant vector).

Schedule (the key to perf): everything is one software-pipelined stream.
  - The projection chunks (fp8 DoubleRow matmuls) are interleaved into the
    attention loop one chunk per (jq,c,si) unit, so TensorE fills the gaps
    while ScalarE runs the exp chain and the first exp starts ~10us in.
  - Within a pair's si loop, scores(si+1) is issued BEFORE z(si): TensorE
    streams the next tile's scores while ScalarE exps tile si, then the z
    matmuls run as soon as the exp lands.  psum: scores ping-pong 2x2 banks,
    z 2x1, out-proj 1, projections 1 = 8 banks exactly.
  - Causal masking of the diagonal 128x128 block: one bf16 triangular-mask
    multiply on the exp output (DVE, which has slack).
  - v carries 64 ones-columns (v' = [1...1 | v]) so the z^T matmul yields
    ps_z[0:64]=denominator replicated 64x and ps_z[64:128]=z^T: the softmax
    normalization is just reciprocal_approx_fast + one tensor_mul, no
    gpsimd partition-broadcast and no row copies.  (The ones block comes
    FIRST because the custom-DVE reciprocal drops the partition offset of
    PSUM sources - it must read from partition base 0.)
"""

import os
import sys

for _p in ("/opt/trn_rl_repo", os.path.expanduser("~/.axon_site/_ro/trn_rl_repo")):
    if os.path.isdir(_p) and _p not in sys.path:
        sys.path.insert(0, _p)

import ml_dtypes
import numpy as np

import concourse.bass as bass
import concourse.mybir as mybir
import concourse.tile as tile
from concourse import bacc
from concourse.bass_utils import run_bass_kernel_spmd

B, S, M, H, D = 2, 2048, 1024, 16, 64
HG = 4                 # heads per core
NCORES = 8
SQ = 512               # sq chunk width (one fp32 psum bank)
NSQ = S // SQ          # 4
NMC = M // 128         # 8 contraction chunks for projections
NSS = S // 128         # 16 s sub-chunks of 128

F8 = mybir.dt.float8e4
BF = mybir.dt.bfloat16
F32 = mybir.dt.float32
EXP = mybir.ActivationFunctionType.Exp
DR = mybir.MatmulPerfMode.DoubleRow

_f8 = ml_dtypes.float8_e4m3
_bf16 = ml_dtypes.bfloat16


def _build_nc():
    nc = bacc.Bacc(
        "TRN2", target_bir_lowering=False,
        debug=os.environ.get("KDEBUG", "0") == "1", num_devices=NCORES
    )

    xq = nc.declare_dram_parameter("xq_t8", [M, S], F8, isOutput=False)
    xk = nc.declare_dram_parameter("xk_t8", [M, S], F8, isOutput=False)
    xv = nc.declare_dram_parameter("xv_t8", [M, S], F8, isOutput=False)
    wqkv = nc.declare_dram_parameter(
        "wqkv8", [128, 3 * NMC * HG * D], F8, isOutput=False
    )
    wo = nc.declare_dram_parameter("wo_bf", [128, 2 * M], BF, isOutput=False)
    bqk = nc.declare_dram_parameter("bqk", [128, 4], F32, isOutput=False)
    out_p = nc.declare_dram_parameter("out_p", [S, M], BF, isOutput=True)
    KDUMP = os.environ.get("KDUMP", "0") == "1"
    if KDUMP:
        dbg_qt = nc.declare_dram_parameter("dbg_qt", [128, 2 * S], BF, isOutput=True)
        dbg_kt = nc.declare_dram_parameter("dbg_kt", [128, 2 * S], BF, isOutput=True)
        dbg_v = nc.declare_dram_parameter(
            "dbg_v", [128, NSS * HG * 2 * D], BF, isOutput=True
        )
        dbg_zt = nc.declare_dram_parameter("dbg_zt", [128, 2 * S], BF, isOutput=True)

    with tile.TileContext(nc) as tc:
        with (
            tc.tile_pool(name="persist", bufs=1) as pers,
            tc.tile_pool(name="work", bufs=6) as work,
            tc.tile_pool(name="ppx", bufs=2, space="PSUM") as ppx,
            tc.tile_pool(name="pps", bufs=2, space="PSUM") as pps,
            tc.tile_pool(name="ppz", bufs=2, space="PSUM") as ppz,
        ):
            # ---- persistent SBUF tensors ----
            xq_sb = pers.tile([128, NMC, S], F8, tag="xq")
            xk_sb = pers.tile([128, NMC, S], F8, tag="xk")
            xv_sb = pers.tile([128, NMC, S], F8, tag="xv")
            wqkv_sb = pers.tile([128, 3, NMC, HG * D], F8, tag="wqkv")
            wq_sb, wk_sb, wv_sb = (wqkv_sb[:, i] for i in range(3))
            wo_sb = pers.tile([128, 2, M], BF, tag="wo")
            bqk_sb = pers.tile([128, 4], F32, tag="bqk")
            bq_sb, bk_sb = bqk_sb[:, 0:2], bqk_sb[:, 2:4]
            qt_sb = pers.tile([128, 2, S], BF, tag="qt")
            kt_sb = pers.tile([128, 2, S], BF, tag="kt")
            zt_sb = pers.tile([128, 2, S], BF, tag="zt")
            # v': per (ss, head) 64 ones cols + 64 data cols
            v_sb = pers.tile([128, NSS, HG, 2 * D], BF, tag="v")
            trimask = pers.tile([128, 128], BF, tag="trimask")

            # ---- constants ----
            # ones block of v' = [1...1 | v]: the denominator lands in psum
            # rows 0:64 (custom-DVE recip mishandles psum partition offsets,
            # so it must read from partition base 0), z in rows 64:128.
            nc.vector.memset(v_sb[:, :, :, 0:D], 1.0)
            # trimask: keep (1.0) where p <= f
            nc.gpsimd.memset(trimask[:, :], 1.0)
            nc.gpsimd.affine_select(
                out=trimask[:, :], in_=trimask[:, :],
                compare_op=mybir.AluOpType.is_ge, fill=0.0,
                base=0, pattern=[[1, 128]], channel_multiplier=-1,
            )
            # warm the exp table set during the DMA phase
            expwarm = pers.tile([1, 1], F32, tag="expwarm")
            nc.scalar.activation(expwarm[:, :], trimask[0:1, 0:1], EXP)
            # warm the PE HAM clock gate (K=4/8 -> 8/8 needs ~3.4us of
            # sustained matmul activity) while TensorE waits on input DMAs
            for _ in range(16):
                wps = ppx.tile([128, SQ], F32, tag="ppx")
                nc.tensor.matmul(
                    wps[:, 0:128], lhsT=trimask[:, :], rhs=trimask[:, :],
                    start=True, stop=True,
                )

            # ---- input DMAs ----
            W1 = NMC * HG * D
            # q,k weights + first-wave activations first (gate the pipeline)
            for i in (0, 1):
                nc.sync.dma_start(
                    out=wqkv_sb[:, i, :, :], in_=wqkv[:, W1 * i : W1 * i + W1]
                )
            nc.sync.dma_start(out=bqk_sb[:, :], in_=bqk[:, :])
            # first waves on the scalar queue: q,k cols 0:512 then 512:1024
            for x_sb, x_dram, q2, ssl0, ssl1 in (
                (xq_sb, xq, 0, 0, 512), (xq_sb, xq, 1, 0, 512),
                (xk_sb, xk, 0, 0, 512), (xk_sb, xk, 1, 0, 512),
                (xq_sb, xq, 0, 512, 1024), (xq_sb, xq, 1, 512, 1024),
                (xk_sb, xk, 0, 512, 1024), (xk_sb, xk, 1, 512, 1024),
            ):
                nc.scalar.dma_start(
                    out=x_sb[:, 4 * q2 : 4 * q2 + 4, ssl0:ssl1],
                    in_=x_dram[512 * q2 : 512 * q2 + 512, ssl0:ssl1].rearrange(
                        "(c p) s -> p c s", p=128
                    ),
                )
            # v weights + v first half, then wo, then second halves
            nc.sync.dma_start(out=wqkv_sb[:, 2, :, :], in_=wqkv[:, 2 * W1 : 3 * W1])
            for q2 in range(2):
                nc.sync.dma_start(
                    out=xv_sb[:, 4 * q2 : 4 * q2 + 4, 0:1024],
                    in_=xv[512 * q2 : 512 * q2 + 512, 0:1024].rearrange(
                        "(c p) s -> p c s", p=128
                    ),
                )
            nc.sync.dma_start(out=wo_sb[:, :, :], in_=wo[:, :])
            for x_sb, x_dram in ((xq_sb, xq), (xk_sb, xk), (xv_sb, xv)):
                for q2 in range(2):
                    nc.sync.dma_start(
                        out=x_sb[:, 4 * q2 : 4 * q2 + 4, 1024:2048],
                        in_=x_dram[512 * q2 : 512 * q2 + 512, 1024:2048].rearrange(
                            "(c p) s -> p c s", p=128
                        ),
                    )

            # ---- chunk generators (issued lazily, interleaved) ----
            def proj_qk(t, half, which):
                def go():
                    ssl = slice(SQ * t, SQ * t + SQ)
                    dsl = slice(128 * half, 128 * half + 128)
                    dst_sb, w_sb, x_sb, b_sb, scale = (
                        (qt_sb, wq_sb, xq_sb, bq_sb, 0.125)
                        if which == "q"
                        else (kt_sb, wk_sb, xk_sb, bk_sb, None)
                    )
                    ps = ppx.tile([128, SQ], F32, tag="ppx")
                    for mi in range(0, NMC, 2):
                        nc.tensor.matmul(
                            ps[:, :],
                            lhsT=w_sb[:, mi : mi + 2, dsl],
                            rhs=x_sb[:, mi : mi + 2, ssl],
                            start=(mi == 0),
                            stop=(mi == NMC - 2),
                            perf_mode=DR,
                        )
                    if scale is None:
                        nc.vector.tensor_scalar_add(
                            dst_sb[:, half, ssl], ps[:, :], b_sb[:, half : half + 1]
                        )
                    else:
                        nc.vector.tensor_scalar(
                            out=dst_sb[:, half, ssl],
                            in0=ps[:, :],
                            scalar1=b_sb[:, half : half + 1],
                            scalar2=scale,
                            op0=mybir.AluOpType.add,
                            op1=mybir.AluOpType.mult,
                        )
                return go

            def proj_v(ss):
                def go():
                    psl = slice(128 * ss, 128 * ss + 128)
                    ps = ppx.tile([128, SQ], F32, tag="ppx")
                    for mi in range(0, NMC, 2):
                        nc.tensor.matmul(
                            ps[:, 0 : HG * D],
                            lhsT=xv_sb[:, mi : mi + 2, psl],
                            rhs=wv_sb[:, mi : mi + 2, :],
                            start=(mi == 0),
                            stop=(mi == NMC - 2),
                            perf_mode=DR,
                        )
                    nc.vector.tensor_copy(
                        v_sb[:, ss, :, D : 2 * D],
                        ps[:, 0 : HG * D].rearrange("p (g d) -> p g d", g=HG),
                    )
                return go

            def out_proj(jq, ss4, n, tail_idx=None):
                def go():
                    psl = slice(SQ * jq + 128 * ss4, SQ * jq + 128 * ss4 + 128)
                    nsl = slice(SQ * n, SQ * n + SQ)
                    ps_o = ppx.tile([128, SQ], F32, tag="ppx")
                    for c2 in range(2):
                        nc.tensor.matmul(
                            ps_o[:, :],
                            lhsT=zt_sb[:, c2, psl],
                            rhs=wo_sb[:, c2, nsl],
                            start=(c2 == 0),
                            stop=(c2 == 1),
                        )
                    o_sb = work.tile([128, SQ], BF, tag="o")
                    nc.vector.tensor_copy(o_sb[:, :], ps_o[:, :])
                    nc.sync.dma_start(out=out_p[psl, nsl], in_=o_sb[:, :])
                return go

            # ---- prologue: projections for window 0, ordered to match DMA
            # arrival (both q chunks run while the k waves still land) ----
            for which in ("q", "k"):
                for half in range(2):
                    proj_qk(0, half, which)()
            for ss in range(4):
                proj_v(ss)()

            # ---- attention: one flat pipelined stream over (jq, c, si) ----
            # scores for unit k+1 are issued before z of unit k, ACROSS pair
            # and window boundaries, so the exp pipeline never drains until
            # the very end.
            def scores(jq, c, si):
                ksl = slice(128 * si, 128 * si + 128)
                r = si - 4 * jq
                w0 = 128 * r if r > 0 else 0
                ps2 = pps.tile([128, 2, SQ], F32, tag="pps")
                for u in range(2):
                    hsl = slice(64 * u, 64 * u + 64)
                    nc.tensor.matmul(
                        ps2[:, u, w0:SQ],
                        lhsT=kt_sb[hsl, c, ksl],
                        rhs=qt_sb[hsl, c, SQ * jq + w0 : SQ * jq + SQ],
                        start=True,
                        stop=True,
                    )
                return ps2

            units = []
            for jq in range(NSQ):
                for c in range(2):
                    for si in range(4 * (jq + 1)):
                        units.append((jq, c, si))

            def make_pending(jq):
                p = []
                if jq < NSQ - 1:
                    for half in range(2):
                        for which in ("q", "k"):
                            p.append(proj_qk(jq + 1, half, which))
                    for ss in range(4 * jq + 4, 4 * jq + 8):
                        p.append(proj_v(ss))
                if jq >= 1:
                    for ss4 in range(4):
                        for n in range(M // SQ):
                            p.append(out_proj(jq - 1, ss4, n))
                return p

            pending = make_pending(0)
            stride = max(1, 8 // max(1, len(pending)))
            unit_in_jq = 0
            psz = None
            cur = scores(*units[0])
            for k, (jq, c, si) in enumerate(units):
                nsk = 4 * (jq + 1)
                qsl = slice(SQ * jq, SQ * jq + SQ)
                r = si - 4 * jq
                w0 = 128 * r if r > 0 else 0
                if si == 0:
                    psz = [
                        ppz.tile([128, SQ], F32, tag="ppz", name=f"psz{jq}_{c}_{u}")
                        for u in range(2)
                    ]
                if si == nsk - 1 and k + 1 < len(units) and units[k + 1][0] != jq:
                    # next unit starts a new window: its scores need that
                    # window's projections, so flush them into the TensorE
                    # queue first (avoids a queue-order deadlock)
                    for go in pending:
                        go()
                    pending = make_pending(jq + 1)
                    stride = max(1, (2 * 4 * (jq + 2)) // max(1, len(pending)))
                    unit_in_jq = -1
                nxt = scores(*units[k + 1]) if k + 1 < len(units) else None
                p_bf = work.tile([128, 2, SQ], BF, tag="p")
                nc.scalar.activation(p_bf[:, :, w0:SQ], cur[:, :, w0:SQ], EXP)
                if r >= 0:
                    # zero the upper-triangular part of the diagonal
                    # 128-block (causal mask) on the idle DVE
                    for u in range(2):
                        nc.vector.tensor_mul(
                            p_bf[:, u, w0 : w0 + 128],
                            p_bf[:, u, w0 : w0 + 128],
                            trimask[:, :],
                        )
                for u in range(2):
                    h = 2 * c + u
                    nc.tensor.matmul(
                        psz[u][:, w0:SQ],
                        lhsT=v_sb[:, si, h, :],
                        rhs=p_bf[:, u, w0:SQ],
                        start=(si == 0),
                        stop=(si == nsk - 1),
                    )
                if pending and unit_in_jq >= 0 and unit_in_jq % stride == 0:
                    pending.pop(0)()
                unit_in_jq += 1
                if si == nsk - 1:
                    # normalize: rows 0:64 of psz hold the denominator,
                    # rows 64:128 hold z^T
                    for u in range(2):
                        rb = work.tile([D, SQ], F32, tag="rb")
                        nc.vector.reciprocal_approx_fast(
                            out=rb[:, :], in_=psz[u][0:D, :]
                        )
                        nc.vector.tensor_mul(
                            zt_sb[64 * u : 64 * u + 64, c, qsl],
                            psz[u][D : 2 * D, :],
                            rb[:, :],
                        )
                cur = nxt

            # ---- tail: output projection for the last window, pipelined
            # 2-deep so the c=0 matmuls (which need only zt[c=0], normalized
            # much earlier) run while the DVE still normalizes c=1 ----
            def tail_c0(ss4, n):
                psl = slice(SQ * (NSQ - 1) + 128 * ss4,
                            SQ * (NSQ - 1) + 128 * ss4 + 128)
                nsl = slice(SQ * n, SQ * n + SQ)
                ps_o = ppx.tile([128, SQ], F32, tag="ppx")
                nc.tensor.matmul(
                    ps_o[:, :], lhsT=zt_sb[:, 0, psl], rhs=wo_sb[:, 0, nsl],
                    start=True, stop=False,
                )
                return ps_o

            def tail_fin(ss4, n, ps_o):
                psl = slice(SQ * (NSQ - 1) + 128 * ss4,
                            SQ * (NSQ - 1) + 128 * ss4 + 128)
                nsl = slice(SQ * n, SQ * n + SQ)
                nc.tensor.matmul(
                    ps_o[:, :], lhsT=zt_sb[:, 1, psl], rhs=wo_sb[:, 1, nsl],
                    start=False, stop=True,
                )
                o_sb = work.tile([128, SQ], BF, tag="o")
                nc.vector.tensor_copy(o_sb[:, :], ps_o[:, :])
                nc.sync.dma_start(out=out_p[psl, nsl], in_=o_sb[:, :])

            tail_pend = []
            for ss4 in range(4):
                for n in range(M // SQ):
                    tail_pend.append((ss4, n, tail_c0(ss4, n)))
                    if len(tail_pend) == 2:
                        tail_fin(*tail_pend.pop(0))
            for t in tail_pend:
                tail_fin(*t)

            if KDUMP:
                nc.sync.dma_start(
                    out=dbg_qt[:, :], in_=qt_sb[:, :, :].rearrange("p a b -> p (a b)")
                )
                nc.sync.dma_start(
                    out=dbg_kt[:, :], in_=kt_sb[:, :, :].rearrange("p a b -> p (a b)")
                )
                nc.sync.dma_start(
                    out=dbg_v[:, :],
                    in_=v_sb[:, :, :, :].rearrange("p a b c -> p (a b c)"),
                )
                nc.sync.dma_start(
                    out=dbg_zt[:, :], in_=zt_sb[:, :, :].rearrange("p a b -> p (a b)")
                )

    if not nc.is_finalized():
        nc.finalize()
    return nc


_NC = None


def _get_nc():
    global _NC
    if _NC is None:
        _NC = _build_nc()
    return _NC


def _wpack(w):
    """[M, HG*D] -> partition-major [128, NMC*HG*D] (2 KiB contiguous rows)."""
    return np.ascontiguousarray(
        w.reshape(NMC, 128, HG * D).transpose(1, 0, 2).reshape(128, NMC * HG * D)
    )


def _make_in_maps(inputs):
    q8 = lambda a: np.asarray(a, np.float32).astype(_f8)
    xt = {}
    for name, key in (("xq_t8", "query_input"), ("xk_t8", "key_input"),
                      ("xv_t8", "value_input")):
        xt[name] = [np.ascontiguousarray(q8(inputs[key][b]).T) for b in range(B)]

    wq8 = q8(inputs["W_Q"])  # [H, M, D]
    wk8 = q8(inputs["W_K"])
    wv8 = q8(inputs["W_V"])
    wo = np.asarray(inputs["W_O"], np.float32)  # [H, D, M]

    in_maps = []
    for core in range(NCORES):
        b, hg = core // HG, core % HG
        hs = slice(HG * hg, HG * hg + HG)
        m = {
            "xq_t8": xt["xq_t8"][b],
            "xk_t8": xt["xk_t8"][b],
            "xv_t8": xt["xv_t8"][b],
            "wqkv8": np.concatenate(
                [
                    _wpack(w[hs].transpose(1, 0, 2).reshape(M, HG * D))
                    for w in (wq8, wk8, wv8)
                ],
                axis=1,
            ),
            "wo_bf": np.ascontiguousarray(
                wo[hs]
                .reshape(HG * D, M)
                .astype(_bf16)
                .reshape(2, 128, M)
                .transpose(1, 0, 2)
                .reshape(128, 2 * M)
            ),
            "bqk": np.ascontiguousarray(
                np.concatenate(
                    [
                        np.asarray(inputs[k], np.float32)[hs].reshape(2, 128).T
                        for k in ("b_Q", "b_K")
                    ],
                    axis=1,
                )
            ),
        }
        in_maps.append(m)
    return in_maps


def _run(inputs, **kw):
    nc = _get_nc()
    in_maps = _make_in_maps(inputs)
    res = run_bass_kernel_spmd(nc, in_maps, list(range(NCORES)), **kw)
    out = np.zeros((B, S, M), np.float32)
    for core in range(NCORES):
        out[core // HG] += np.asarray(res.results[core]["out_p"], np.float32)
    # V-bias folds to a constant vector (softmax rows sum to 1): b_V @ W_O
    bv = np.asarray(inputs["b_V"], np.float32)          # [H, D]
    wo = np.asarray(inputs["W_O"], np.float32)          # [H, D, M]
    out += np.einsum("hd,hdm->m", bv, wo) + np.asarray(inputs["b_O"], np.float32)
    return out, res


def kernel(**inputs):
    out, _ = _run(inputs)
    return out


# revision 56
# speedup vs baseline: 1.0093x; 1.0093x over previous
"""Trainium2 Bass/Tile kernel: fused fp8-quantized multi-head causal attention.

Module: q/k/v = fp8(x) @ fp8(W) + b ; scores = (q k^T)/sqrt(64) with causal
mask (-1000 => exp underflows to exactly 0) ; out = softmax(scores) @ v @ W_O + b_O.

Sharding (8 NeuronCores, SPMD, no collectives):
  core c -> batch b = c // 4, head group hg = c % 4 (heads 4*hg .. 4*hg+3).
  Each core returns a partial [S, M] bf16 output (its 4 heads' contribution);
  the host sums the 4 partials per batch in f32 and adds b_O + sum_h b_V[h]@W_O[h]
  (exact: softmax rows sum to 1, so the V bias contributes a constant vector).

Schedule (the key to perf): everything is one software-pipelined stream.
  - The projection chunks (fp8 DoubleRow matmuls) and the previous window's
    output-projection tiles are interleaved into the attention loop, one
    chunk per (jq,c,si) unit, so TensorE fills the exp-wait gaps.
  - The attention is one flat unit stream over (jq, c, si): scores(unit k+1)
    are issued BEFORE z(unit k), across pair and window boundaries, so
    TensorE streams the next tile's scores while ScalarE exps tile k.
    psum: scores ping-pong 2x2 banks, z 2x1, shared proj/out-proj
    ping-pong 2x1 = 8 banks exactly.  The tail out-projection is pipelined
    2-deep on the shared pool.
  - Causal masking of the diagonal 128x128 block: one bf16 triangular-mask
    multiply on the exp output (DVE, which has slack).
  - v carries 64 ones-columns (v' = [1...1 | v]) so the z^T matmul yields
    ps_z[0:64]=denominator replicated 64x and ps_z[64:128]=z^T: the softmax
    normalization is just reciprocal_approx_fast + one tensor_mul, no
    gpsimd partition-broadcast and no row copies.  (The ones block comes
    FIRST because the custom-DVE reciprocal drops the partition offset of
    PSUM sources - it must read from partition base 0.)
"""

import os
import sys

for _p in ("/opt/trn_rl_repo", os.path.expanduser("~/.axon_site/_ro/trn_rl_repo")):
    if os.path.isdir(_p) and _p not in sys.path:
        sys.path.insert(0, _p)

import ml_dtypes
import numpy as np

import concourse.bass as bass
import concourse.mybir as mybir
import concourse.tile as tile
from concourse import bacc
from concourse.bass_utils import run_bass_kernel_spmd

B, S, M, H, D = 2, 2048, 1024, 16, 64
HG = 4                 # heads per core
NCORES = 8
SQ = 512               # sq chunk width (one fp32 psum bank)
NSQ = S // SQ          # 4
NMC = M // 128         # 8 contraction chunks for projections
NSS = S // 128         # 16 s sub-chunks of 128

F8 = mybir.dt.float8e4
BF = mybir.dt.bfloat16
F32 = mybir.dt.float32
EXP = mybir.ActivationFunctionType.Exp
DR = mybir.MatmulPerfMode.DoubleRow

_f8 = ml_dtypes.float8_e4m3
_bf16 = ml_dtypes.bfloat16


def _build_nc():
    nc = bacc.Bacc(
        "TRN2", target_bir_lowering=False,
        debug=os.environ.get("KDEBUG", "0") == "1", num_devices=NCORES
    )

    xq = nc.declare_dram_parameter("xq_t8", [M, S], F8, isOutput=False)
    xk = nc.declare_dram_parameter("xk_t8", [M, S], F8, isOutput=False)
    xv = nc.declare_dram_parameter("xv_t8", [M, S], F8, isOutput=False)
    wqkv = nc.declare_dram_parameter(
        "wqkv8", [128, 3 * NMC * HG * D], F8, isOutput=False
    )
    wo = nc.declare_dram_parameter("wo_bf", [128, 2 * M], BF, isOutput=False)
    bqk = nc.declare_dram_parameter("bqk", [128, 4], F32, isOutput=False)
    out_p = nc.declare_dram_parameter("out_p", [S, M], BF, isOutput=True)
    KDUMP = os.environ.get("KDUMP", "0") == "1"
    if KDUMP:
        dbg_qt = nc.declare_dram_parameter("dbg_qt", [128, 2 * S], BF, isOutput=True)
        dbg_kt = nc.declare_dram_parameter("dbg_kt", [128, 2 * S], BF, isOutput=True)
        dbg_v = nc.declare_dram_parameter(
            "dbg_v", [128, NSS * HG * 2 * D], BF, isOutput=True
        )
        dbg_zt = nc.declare_dram_parameter("dbg_zt", [128, 2 * S], BF, isOutput=True)

    with tile.TileContext(nc) as tc:
        with (
            tc.tile_pool(name="persist", bufs=1) as pers,
            tc.tile_pool(name="work", bufs=6) as work,
            tc.tile_pool(name="ppx", bufs=2, space="PSUM") as ppx,
            tc.tile_pool(name="pps", bufs=2, space="PSUM") as pps,
            tc.tile_pool(name="ppz", bufs=2, space="PSUM") as ppz,
        ):
            # ---- persistent SBUF tensors ----
            xq_sb = pers.tile([128, NMC, S], F8, tag="xq")
            xk_sb = pers.tile([128, NMC, S], F8, tag="xk")
            xv_sb = pers.tile([128, NMC, S], F8, tag="xv")
            wqkv_sb = pers.tile([128, 3, NMC, HG * D], F8, tag="wqkv")
            wq_sb, wk_sb, wv_sb = (wqkv_sb[:, i] for i in range(3))
            wo_sb = pers.tile([128, 2, M], BF, tag="wo")
            bqk_sb = pers.tile([128, 4], F32, tag="bqk")
            bq_sb, bk_sb = bqk_sb[:, 0:2], bqk_sb[:, 2:4]
            qt_sb = pers.tile([128, 2, S], BF, tag="qt")
            kt_sb = pers.tile([128, 2, S], BF, tag="kt")
            zt_sb = pers.tile([128, 2, S], BF, tag="zt")
            # v': per (ss, head) 64 ones cols + 64 data cols
            v_sb = pers.tile([128, NSS, HG, 2 * D], BF, tag="v")
            trimask = pers.tile([128, 128], BF, tag="trimask")

            # ---- constants ----
            # ones block of v' = [1...1 | v]: the denominator lands in psum
            # rows 0:64 (custom-DVE recip mishandles psum partition offsets,
            # so it must read from partition base 0), z in rows 64:128.
            nc.vector.memset(v_sb[:, :, :, 0:D], 1.0)
            # trimask: keep (1.0) where p <= f
            nc.gpsimd.memset(trimask[:, :], 1.0)
            nc.gpsimd.affine_select(
                out=trimask[:, :], in_=trimask[:, :],
                compare_op=mybir.AluOpType.is_ge, fill=0.0,
                base=0, pattern=[[1, 128]], channel_multiplier=-1,
            )
            # warm the exp table set during the DMA phase
            expwarm = pers.tile([1, 1], F32, tag="expwarm")
            nc.scalar.activation(expwarm[:, :], trimask[0:1, 0:1], EXP)
            # warm the PE HAM clock gate (K=4/8 -> 8/8 needs ~3.4us of
            # sustained matmul activity) while TensorE waits on input DMAs
            for _ in range(16):
                wps = ppx.tile([128, SQ], F32, tag="ppx")
                nc.tensor.matmul(
                    wps[:, 0:128], lhsT=trimask[:, :], rhs=trimask[:, :],
                    start=True, stop=True,
                )

            # ---- input DMAs ----
            W1 = NMC * HG * D
            # q,k weights + first-wave activations first (gate the pipeline)
            for i in (0, 1):
                nc.sync.dma_start(
                    out=wqkv_sb[:, i, :, :], in_=wqkv[:, W1 * i : W1 * i + W1]
                )
            nc.sync.dma_start(out=bqk_sb[:, :], in_=bqk[:, :])
            # first waves on the scalar queue: q,k cols 0:512 then 512:1024
            for x_sb, x_dram, q2, ssl0, ssl1 in (
                (xq_sb, xq, 0, 0, 512), (xq_sb, xq, 1, 0, 512),
                (xk_sb, xk, 0, 0, 512), (xk_sb, xk, 1, 0, 512),
                (xq_sb, xq, 0, 512, 1024), (xq_sb, xq, 1, 512, 1024),
                (xk_sb, xk, 0, 512, 1024), (xk_sb, xk, 1, 512, 1024),
            ):
                nc.scalar.dma_start(
                    out=x_sb[:, 4 * q2 : 4 * q2 + 4, ssl0:ssl1],
                    in_=x_dram[512 * q2 : 512 * q2 + 512, ssl0:ssl1].rearrange(
                        "(c p) s -> p c s", p=128
                    ),
                )
            # v weights + v first half, then wo, then second halves
            nc.sync.dma_start(out=wqkv_sb[:, 2, :, :], in_=wqkv[:, 2 * W1 : 3 * W1])
            for q2 in range(2):
                nc.sync.dma_start(
                    out=xv_sb[:, 4 * q2 : 4 * q2 + 4, 0:1024],
                    in_=xv[512 * q2 : 512 * q2 + 512, 0:1024].rearrange(
                        "(c p) s -> p c s", p=128
                    ),
                )
            nc.sync.dma_start(out=wo_sb[:, :, :], in_=wo[:, :])
            for x_sb, x_dram in ((xq_sb, xq), (xk_sb, xk), (xv_sb, xv)):
                for q2 in range(2):
                    nc.sync.dma_start(
                        out=x_sb[:, 4 * q2 : 4 * q2 + 4, 1024:2048],
                        in_=x_dram[512 * q2 : 512 * q2 + 512, 1024:2048].rearrange(
                            "(c p) s -> p c s", p=128
                        ),
                    )

            # ---- chunk generators (issued lazily, interleaved) ----
            def proj_qk(t, half, which):
                def go():
                    ssl = slice(SQ * t, SQ * t + SQ)
                    dsl = slice(128 * half, 128 * half + 128)
                    dst_sb, w_sb, x_sb, b_sb, scale = (
                        (qt_sb, wq_sb, xq_sb, bq_sb, 0.125)
                        if which == "q"
                        else (kt_sb, wk_sb, xk_sb, bk_sb, None)
                    )
                    ps = ppx.tile([128, SQ], F32, tag="ppx")
                    for mi in range(0, NMC, 2):
                        nc.tensor.matmul(
                            ps[:, :],
                            lhsT=w_sb[:, mi : mi + 2, dsl],
                            rhs=x_sb[:, mi : mi + 2, ssl],
                            start=(mi == 0),
                            stop=(mi == NMC - 2),
                            perf_mode=DR,
                        )
                    if scale is None:
                        nc.vector.tensor_scalar_add(
                            dst_sb[:, half, ssl], ps[:, :], b_sb[:, half : half + 1]
                        )
                    else:
                        nc.vector.tensor_scalar(
                            out=dst_sb[:, half, ssl],
                            in0=ps[:, :],
                            scalar1=b_sb[:, half : half + 1],
                            scalar2=scale,
                            op0=mybir.AluOpType.add,
                            op1=mybir.AluOpType.mult,
                        )
                return go

            def proj_v(ss):
                def go():
                    psl = slice(128 * ss, 128 * ss + 128)
                    ps = ppx.tile([128, SQ], F32, tag="ppx")
                    for mi in range(0, NMC, 2):
                        nc.tensor.matmul(
                            ps[:, 0 : HG * D],
                            lhsT=xv_sb[:, mi : mi + 2, psl],
                            rhs=wv_sb[:, mi : mi + 2, :],
                            start=(mi == 0),
                            stop=(mi == NMC - 2),
                            perf_mode=DR,
                        )
                    nc.vector.tensor_copy(
                        v_sb[:, ss, :, D : 2 * D],
                        ps[:, 0 : HG * D].rearrange("p (g d) -> p g d", g=HG),
                    )
                return go

            def out_proj(jq, ss4, n, tail_idx=None):
                def go():
                    psl = slice(SQ * jq + 128 * ss4, SQ * jq + 128 * ss4 + 128)
                    nsl = slice(SQ * n, SQ * n + SQ)
                    ps_o = ppx.tile([128, SQ], F32, tag="ppx")
                    for c2 in range(2):
                        nc.tensor.matmul(
                            ps_o[:, :],
                            lhsT=zt_sb[:, c2, psl],
                            rhs=wo_sb[:, c2, nsl],
                            start=(c2 == 0),
                            stop=(c2 == 1),
                        )
                    o_sb = work.tile([128, SQ], BF, tag="o")
                    nc.vector.tensor_copy(o_sb[:, :], ps_o[:, :])
                    nc.sync.dma_start(out=out_p[psl, nsl], in_=o_sb[:, :])
                return go

            # ---- prologue: projections for window 0, ordered to match DMA
            # arrival (both q chunks run while the k waves still land) ----
            for which in ("q", "k"):
                for half in range(2):
                    proj_qk(0, half, which)()
            for ss in range(4):
                proj_v(ss)()

            # ---- attention: one flat pipelined stream over (jq, c, si) ----
            # scores for unit k+1 are issued before z of unit k, ACROSS pair
            # and window boundaries, so the exp pipeline never drains until
            # the very end.
            def scores(jq, c, si):
                ksl = slice(128 * si, 128 * si + 128)
                r = si - 4 * jq
                w0 = 128 * r if r > 0 else 0
                ps2 = pps.tile([128, 2, SQ], F32, tag="pps")
                for u in range(2):
                    hsl = slice(64 * u, 64 * u + 64)
                    nc.tensor.matmul(
                        ps2[:, u, w0:SQ],
                        lhsT=kt_sb[hsl, c, ksl],
                        rhs=qt_sb[hsl, c, SQ * jq + w0 : SQ * jq + SQ],
                        start=True,
                        stop=True,
                    )
                return ps2

            units = []
            for jq in range(NSQ):
                for c in range(2):
                    for si in range(4 * (jq + 1)):
                        units.append((jq, c, si))

            def make_pending(jq):
                p = []
                if jq < NSQ - 1:
                    for half in range(2):
                        for which in ("q", "k"):
                            p.append(proj_qk(jq + 1, half, which))
                    for ss in range(4 * jq + 4, 4 * jq + 8):
                        p.append(proj_v(ss))
                if jq >= 1:
                    for ss4 in range(4):
                        for n in range(M // SQ):
                            p.append(out_proj(jq - 1, ss4, n))
                return p

            pending = make_pending(0)
            stride = max(1, 8 // max(1, len(pending)))
            unit_in_jq = 0
            psz = None
            cur = scores(*units[0])
            for k, (jq, c, si) in enumerate(units):
                nsk = 4 * (jq + 1)
                qsl = slice(SQ * jq, SQ * jq + SQ)
                r = si - 4 * jq
                w0 = 128 * r if r > 0 else 0
                if si == 0:
                    psz = [
                        ppz.tile([128, SQ], F32, tag="ppz", name=f"psz{jq}_{c}_{u}")
                        for u in range(2)
                    ]
                if si == nsk - 1 and k + 1 < len(units) and units[k + 1][0] != jq:
                    # next unit starts a new window: its scores need that
                    # window's projections, so flush them into the TensorE
                    # queue first (avoids a queue-order deadlock)
                    for go in pending:
                        go()
                    pending = make_pending(jq + 1)
                    stride = max(1, (2 * 4 * (jq + 2)) // max(1, len(pending)))
                    unit_in_jq = -1
                nxt = scores(*units[k + 1]) if k + 1 < len(units) else None
                p_bf = work.tile([128, 2, SQ], BF, tag="p")
                nc.scalar.activation(p_bf[:, :, w0:SQ], cur[:, :, w0:SQ], EXP)
                if r >= 0:
                    # zero the upper-triangular part of the diagonal
                    # 128-block (causal mask) on the idle DVE
                    for u in range(2):
                        nc.vector.tensor_mul(
                            p_bf[:, u, w0 : w0 + 128],
                            p_bf[:, u, w0 : w0 + 128],
                            trimask[:, :],
                        )
                for u in range(2):
                    h = 2 * c + u
                    nc.tensor.matmul(
                        psz[u][:, w0:SQ],
                        lhsT=v_sb[:, si, h, :],
                        rhs=p_bf[:, u, w0:SQ],
                        start=(si == 0),
                        stop=(si == nsk - 1),
                    )
                if pending and unit_in_jq >= 0 and unit_in_jq % stride == 0:
                    pending.pop(0)()
                unit_in_jq += 1
                if si == nsk - 1:
                    # normalize: rows 0:64 of psz hold the denominator,
                    # rows 64:128 hold z^T
                    for u in range(2):
                        rb = work.tile([D, SQ], F32, tag="rb")
                        nc.vector.reciprocal_approx_fast(
                            out=rb[:, :], in_=psz[u][0:D, :]
                        )
                        nc.vector.tensor_mul(
                            zt_sb[64 * u : 64 * u + 64, c, qsl],
                            psz[u][D : 2 * D, :],
                            rb[:, :],
                        )
                cur = nxt

            # ---- tail: output projection for the last window, pipelined
            # 2-deep so the c=0 matmuls (which need only zt[c=0], normalized
            # much earlier) run while the DVE still normalizes c=1 ----
            def tail_c0(ss4, n):
                psl = slice(SQ * (NSQ - 1) + 128 * ss4,
                            SQ * (NSQ - 1) + 128 * ss4 + 128)
                nsl = slice(SQ * n, SQ * n + SQ)
                ps_o = ppx.tile([128, SQ], F32, tag="ppx")
                nc.tensor.matmul(
                    ps_o[:, :], lhsT=zt_sb[:, 0, psl], rhs=wo_sb[:, 0, nsl],
                    start=True, stop=False,
                )
                return ps_o

            def tail_fin(ss4, n, ps_o):
                psl = slice(SQ * (NSQ - 1) + 128 * ss4,
                            SQ * (NSQ - 1) + 128 * ss4 + 128)
                nsl = slice(SQ * n, SQ * n + SQ)
                nc.tensor.matmul(
                    ps_o[:, :], lhsT=zt_sb[:, 1, psl], rhs=wo_sb[:, 1, nsl],
                    start=False, stop=True,
                )
                o_sb = work.tile([128, SQ], BF, tag="o")
                nc.vector.tensor_copy(o_sb[:, :], ps_o[:, :])
                nc.sync.dma_start(out=out_p[psl, nsl], in_=o_sb[:, :])

            tail_pend = []
            for ss4 in range(4):
                for n in range(M // SQ):
                    tail_pend.append((ss4, n, tail_c0(ss4, n)))
                    if len(tail_pend) == 2:
                        tail_fin(*tail_pend.pop(0))
            for t in tail_pend:
                tail_fin(*t)

            if KDUMP:
                nc.sync.dma_start(
                    out=dbg_qt[:, :], in_=qt_sb[:, :, :].rearrange("p a b -> p (a b)")
                )
                nc.sync.dma_start(
                    out=dbg_kt[:, :], in_=kt_sb[:, :, :].rearrange("p a b -> p (a b)")
                )
                nc.sync.dma_start(
                    out=dbg_v[:, :],
                    in_=v_sb[:, :, :, :].rearrange("p a b c -> p (a b c)"),
                )
                nc.sync.dma_start(
                    out=dbg_zt[:, :], in_=zt_sb[:, :, :].rearrange("p a b -> p (a b)")
                )

    if not nc.is_finalized():
        nc.finalize()
    return nc


_NC = None


def _get_nc():
    global _NC
    if _NC is None:
        _NC = _build_nc()
    return _NC


def _wpack(w):
    """[M, HG*D] -> partition-major [128, NMC*HG*D] (2 KiB contiguous rows)."""
    return np.ascontiguousarray(
        w.reshape(NMC, 128, HG * D).transpose(1, 0, 2).reshape(128, NMC * HG * D)
    )


def _make_in_maps(inputs):
    q8 = lambda a: np.asarray(a, np.float32).astype(_f8)
    xt = {}
    for name, key in (("xq_t8", "query_input"), ("xk_t8", "key_input"),
                      ("xv_t8", "value_input")):
        xt[name] = [np.ascontiguousarray(q8(inputs[key][b]).T) for b in range(B)]

    wq8 = q8(inputs["W_Q"])  # [H, M, D]
    wk8 = q8(inputs["W_K"])
    wv8 = q8(inputs["W_V"])
    wo = np.asarray(inputs["W_O"], np.float32)  # [H, D, M]

    in_maps = []
    for core in range(NCORES):
        b, hg = core // HG, core % HG
        hs = slice(HG * hg, HG * hg + HG)
        m = {
            "xq_t8": xt["xq_t8"][b],
            "xk_t8": xt["xk_t8"][b],
            "xv_t8": xt["xv_t8"][b],
            "wqkv8": np.concatenate(
                [
                    _wpack(w[hs].transpose(1, 0, 2).reshape(M, HG * D))
                    for w in (wq8, wk8, wv8)
                ],
                axis=1,
            ),
            "wo_bf": np.ascontiguousarray(
                wo[hs]
                .reshape(HG * D, M)
                .astype(_bf16)
                .reshape(2, 128, M)
                .transpose(1, 0, 2)
                .reshape(128, 2 * M)
            ),
            "bqk": np.ascontiguousarray(
                np.concatenate(
                    [
                        np.asarray(inputs[k], np.float32)[hs].reshape(2, 128).T
                        for k in ("b_Q", "b_K")
                    ],
                    axis=1,
                )
            ),
        }
        in_maps.append(m)
    return in_maps


def _run(inputs, **kw):
    nc = _get_nc()
    in_maps = _make_in_maps(inputs)
    res = run_bass_kernel_spmd(nc, in_maps, list(range(NCORES)), **kw)
    out = np.zeros((B, S, M), np.float32)
    for core in range(NCORES):
        out[core // HG] += np.asarray(res.results[core]["out_p"], np.float32)
    # V-bias folds to a constant vector (softmax rows sum to 1): b_V @ W_O
    bv = np.asarray(inputs["b_V"], np.float32)          # [H, D]
    wo = np.asarray(inputs["W_O"], np.float32)          # [H, D, M]
    out += np.einsum("hd,hdm->m", bv, wo) + np.asarray(inputs["b_O"], np.float32)
    return out, res


def kernel(**inputs):
    out, _ = _run(inputs)
    return out
